# revision 6
# baseline (speedup 1.0000x reference)
"""KDTree-distance-loss kernel for Trainium2 (8 NeuronCores, SPMD).

Math: for each src point s (16384 x 3), find min over tgt t (16384 x 3) of
||s-t||^2, clamp (>1.0 -> 0), mean.

Device strategy (data-parallel over src, tgt replicated):
  q[n, m] = -2 s_n . t_m + |t_m|^2   computed on the PE as a K-row matmul
            with augmented operands.  min_m d2 = max(min_m q + |s_n|^2, 0)
            since max(.,0) is monotone and |s|^2 is constant over m.
  The min over m runs on the DVE via tensor_tensor_scan(op0=min, op1=min),
  which consumes TWO fresh 1024-wide spans per instruction (one directly
  from PSUM, one staged to SBUF by the scalar engine) with a running-min
  state chained across scans -- half the DVE element touches of a plain
  running tensor_tensor min.
  Final +|s|^2, clamp and mean run on host (the "all-reduce").
"""

import numpy as np

import concourse.bacc as bacc
import concourse.bass as bass
import concourse.mybir as mybir
from concourse.tile import TileContext

N_CORES = 8
P = 128                      # partitions / src points per block
N_FULL = 16384               # total src points
M_FULL = 16384               # total tgt points
N_PER_CORE = N_FULL // N_CORES          # 2048
NB_FULL = N_PER_CORE // P               # 16 blocks per core
CHUNK = 512                  # matmul moving free dim (one PSUM bank, fp32)
SPAN = 1024                  # scan span (2 PSUM banks)

# "f32r": K=4 float32r operands (full-rate fp32 path; HW numerics TBD)
# "hilo": K=11 float16 hi/lo-split operands (robust ~1e-5 abs accuracy)
VARIANT = "hilo"

_CACHE = {}


def _variant_kdt(variant):
    if variant == "f32r":
        return 4, mybir.dt.float32r
    if variant == "f32":
        return 4, mybir.dt.float32
    if variant == "hilo":
        return 11, mybir.dt.float16
    raise ValueError(variant)


def build(variant=VARIANT, nb=NB_FULL, m=M_FULL):
    K, DT = _variant_kdt(variant)
    n_per_core = nb * P
    gens = m // (2 * SPAN)
    assert m % (2 * SPAN) == 0

    nc = bacc.Bacc(None)
    src_aug = nc.declare_dram_parameter("src_aug", [K, n_per_core], DT, isOutput=False)
    tgt_aug = nc.declare_dram_parameter("tgt_aug", [K, m], DT, isOutput=False)
    out = nc.declare_dram_parameter("out", [P, nb], mybir.dt.float32, isOutput=True)

    f32 = mybir.dt.float32
    MIN = mybir.AluOpType.min

    # Two independent per-block scan chains interleaved on the DVE: hides the
    # per-scan RAW latency of the running-min state chain (sim: 191 -> 162us).
    NWAY = 2
    with TileContext(nc) as tc:
        with (
            tc.tile_pool(name="const", bufs=1) as const_pool,
            tc.tile_pool(name="psumA", bufs=2, space="PSUM") as pA_pool,
            tc.tile_pool(name="psumB", bufs=2, space="PSUM") as pB_pool,
            tc.tile_pool(name="copy", bufs=4) as copy_pool,
            tc.tile_pool(name="scan", bufs=4) as scan_pool,
        ):
            lhs = const_pool.tile([K, n_per_core], DT, tag="lhs")
            nc.sync.dma_start(lhs[:, :], src_aug[:, :])
            rhs = const_pool.tile([K, m], DT, tag="rhs")
            n_dma = 4
            step = m // n_dma
            for i in range(n_dma):
                nc.sync.dma_start(
                    rhs[:, i * step : (i + 1) * step],
                    tgt_aug[:, i * step : (i + 1) * step],
                )
            res = const_pool.tile([P, nb], f32, tag="res")

            for bg in range(0, nb, NWAY):
                prevs = [None] * NWAY
                for g in range(gens):
                    off = g * 2 * SPAN
                    for j in range(NWAY):
                        b = bg + j
                        w = lhs[:, b * P : (b + 1) * P]
                        pA = pA_pool.tile([P, SPAN], f32)
                        pB = pB_pool.tile([P, SPAN], f32)
                        for c in range(SPAN // CHUNK):
                            nc.tensor.matmul(
                                pA[:, c * CHUNK : (c + 1) * CHUNK], w,
                                rhs[:, off + c * CHUNK : off + (c + 1) * CHUNK],
                                start=True, stop=True,
                            )
                        for c in range(SPAN // CHUNK):
                            nc.tensor.matmul(
                                pB[:, c * CHUNK : (c + 1) * CHUNK], w,
                                rhs[:, off + SPAN + c * CHUNK : off + SPAN + (c + 1) * CHUNK],
                                start=True, stop=True,
                            )
                        cB = copy_pool.tile([P, SPAN], f32)
                        nc.scalar.copy(cB[:, :], pB[:, :])
                        so = scan_pool.tile([P, SPAN], f32)
                        init = 3.0e38 if prevs[j] is None else prevs[j][:, SPAN - 1 : SPAN]
                        nc.vector.tensor_tensor_scan(
                            out=so[:, :], data0=pA[:, :], data1=cB[:, :],
                            initial=init, op0=MIN, op1=MIN,
                        )
                        prevs[j] = so
                for j in range(NWAY):
                    nc.scalar.copy(res[:, bg + j : bg + j + 1], prevs[j][:, SPAN - 1 : SPAN])
            nc.sync.dma_start(out[:, :], res[:, :])
    nc.compile()
    return nc


def _prep_aug(src, tgt, variant):
    """Host-side augmentation. Returns (lhsT_full [K, N], rhs [K, M]) numpy."""
    src = np.asarray(src, np.float32)
    tgt = np.asarray(tgt, np.float32)
    n = src.shape[0]
    m = tgt.shape[0]
    u = (-2.0 * tgt.astype(np.float64)).astype(np.float32)  # tgt side, folded -2
    t2 = (tgt.astype(np.float64) ** 2).sum(1).astype(np.float32)
    if variant in ("f32r", "f32"):
        lhsT = np.empty((4, n), np.float32)
        lhsT[0:3] = src.T
        lhsT[3] = 1.0
        rhs = np.empty((4, m), np.float32)
        rhs[0:3] = u.T
        rhs[3] = t2
        return lhsT, rhs
    # hilo float16 split: x = hi + lo with hi = f16(x), lo = f16(x - hi)
    hs = src.astype(np.float16)
    ls = (src - hs.astype(np.float32)).astype(np.float16)
    hu = u.astype(np.float16)
    lu = (u - hu.astype(np.float32)).astype(np.float16)
    t2h = t2.astype(np.float16)
    t2l = (t2 - t2h.astype(np.float32)).astype(np.float16)
    lhsT = np.empty((11, n), np.float16)
    lhsT[0:3] = hs.T
    lhsT[3:6] = ls.T
    lhsT[6:9] = hs.T
    lhsT[9] = np.float16(1.0)
    lhsT[10] = np.float16(1.0)
    rhs = np.empty((11, m), np.float16)
    rhs[0:3] = hu.T
    rhs[3:6] = hu.T
    rhs[6:9] = lu.T
    rhs[9] = t2h
    rhs[10] = t2l
    return lhsT, rhs


def _get_nc(variant):
    key = ("nc", variant)
    if key not in _CACHE:
        _CACHE[key] = build(variant)
    return _CACHE[key]


def _run_device(src, tgt, variant, trace=False):
    from concourse.bass_utils import run_bass_kernel_spmd

    lhsT, rhs = _prep_aug(src, tgt, variant)
    in_maps = []
    for c in range(N_CORES):
        sl = lhsT[:, c * N_PER_CORE : (c + 1) * N_PER_CORE]
        in_maps.append({
            "src_aug": np.ascontiguousarray(sl),
            "tgt_aug": rhs,
        })
    nc = _get_nc(variant)
    r = run_bass_kernel_spmd(nc, in_maps, list(range(N_CORES)), trace=trace)
    # out[c] is [P, NB]; src index within core = b*P + p -> out.T.ravel()
    minq = np.concatenate([r.results[c]["out"].T.ravel() for c in range(N_CORES)])
    return minq, r


def _finish(minq, src):
    src = np.asarray(src, np.float32)
    s2 = (src.astype(np.float64) ** 2).sum(1).astype(np.float32)
    d2 = np.maximum(minq + s2, 0.0)
    clamped = np.where(d2 > 1.0, 0.0, d2)
    return np.float32(clamped.mean(dtype=np.float64))


def kernel(src, tgt, idx=None, **_ignored):
    minq, _ = _run_device(src, tgt, VARIANT, trace=False)
    return np.asarray(_finish(minq, src))


def kernel_traced(src, tgt, idx=None, variant=VARIANT):
    minq, r = _run_device(src, tgt, variant, trace=True)
    return np.asarray(_finish(minq, src)), r
